# revision 59
# baseline (speedup 1.0000x reference)
"""GAT (2-layer graph attention network) on 8 Trainium2 NeuronCores.

v3 strategy (1D node partition, zero L1 collectives):
  Each core recomputes the FULL Wh = X @ W1 for all N nodes locally
  (~28us of PE time) instead of AllGather-ing per-shard Wh: each AllGather
  costs ~15us fixed + bytes/40GBps and they serialize, so the four L1
  gathers (~170us) dominated the v2 runtime.

  Scores use the rank-1 max factorization of exp(leaky_relu(s1_i + s2_j))
  (divide column i by exp(a*s1_i)):
      z[j,i] = Q_j * max(G_i, r_j) * M[j,i]
  with G_i = exp((1-a)s1_i), r_j = exp(-(1-a)s2_j), Q_j = exp(s2_j), and a
  {0,1} adjacency mask M. Per j-tile this is one DVE tensor_scalar (4x perf
  mode) plus a group-batched mask tensor_tensor (2x mode; a slice of groups
  runs on GPSIMD to balance engines).

  Aggregation matmuls put local nodes i on PSUM partitions: stationary =
  z-chunk [128 j, 128 i], moving = [Wh_h | 1] (129 cols; Wh is stored with
  an interleaved ones column per head). The softmax denominator thus rides
  the same matmul as output column 128 — half the PE cost of v2's separate
  512-row denominator pass — and normalize becomes a per-partition scalar
  multiply (no partition_broadcast of reciprocals).

  G_i rows come from a tiny dedicated projection of the core's own X shard
  so the global Wh sweep needs no per-core tile ordering. Layer 2 gathers
  the W2-projected h (64 values + ones + s1'/s2' columns) in ONE AllGather,
  aggregates the same way (gathered ones column = denominator), and writes
  node-major output directly (no final transposes).

Numerics: matmuls bf16 (f32 PSUM), mask exact {0,1}, z products bf16.
"""

import math
from contextlib import ExitStack
from dataclasses import dataclass

import numpy as np
import ml_dtypes

import concourse.bass as bass
import concourse.mybir as mybir
import concourse.tile as tile
from concourse import bacc
from concourse.bass_utils import run_bass_kernel_spmd

BF16 = ml_dtypes.bfloat16
ALPHA = 0.2

# --------------------------------------------------------------------------
# Custom fused DVE op for elu (registered into concourse.dve_ops at import)
# --------------------------------------------------------------------------

import concourse.dve_ops as dve_ops
from concourse.dve_spec import (
    Spec, Src0, Src1, C0, Zero, lower, select, _has_src1,
)
from concourse.dve_uop import DveOpSpec


def _make_elu_spec():
    # out = in0 > 0 ? in0 : in1 - s0   (elu with in1 = exp(in0), s0 = 1.0)
    def _elu_ref(in0, in1, s0, s1, imm2):
        x = in0.astype(np.float32)
        return np.where(x > 0, x, in1.astype(np.float32) - s0)

    return Spec(body=select(Src0 > Zero, Src0, Src1 - C0), reference=_elu_ref)


def _register(name, spec):
    if name in dve_ops._SUB_OPCODE_FOR_NAME:
        for op in dve_ops.OPS:
            if op.name == name:
                return op
    row = max(dve_ops._SUB_OPCODE_FOR_NAME.values()) + 1
    assert row < 0x20
    shas = {}
    for ver in ("v3", "v4"):
        uops = lower(spec, ver=ver)
        shas[ver] = DveOpSpec(
            name=name, opcode=row, uops=uops, rd1_en=_has_src1(spec)
        ).sha(ver)
    op = dve_ops.DveOp(name, spec, subdim=False, uops_sha=shas)
    dve_ops.OPS.append(op)
    dve_ops.CUSTOM_DVE_SPECS[name] = spec
    dve_ops._SUB_OPCODE_FOR_NAME[name] = row
    return op


ELU_SEL = _register("ELU_SEL_GAT", _make_elu_spec())


# --------------------------------------------------------------------------
# Kernel configuration
# --------------------------------------------------------------------------

@dataclass(frozen=True)
class Cfg:
    N: int = 4096      # nodes
    C: int = 512       # input feature dim
    H: int = 128       # hidden per head (must be 128)
    HEADS: int = 4
    F2: int = 64       # output dim
    CORES: int = 8
    GRP: int = 4       # j-tiles per batched mask multiply

    @property
    def R(self): return self.N // self.CORES          # rows per core
    @property
    def JT(self): return self.N // 128                # j tiles
    @property
    def CT(self): return self.C // 128                # input-feature tiles
    @property
    def HH(self): return self.HEADS * self.H          # layer-1 out features
    @property
    def CT2(self): return self.HH // 128              # layer-2 contraction tiles
    @property
    def RT(self): return self.R // 128                # local row tiles
    @property
    def S8(self): return 2 * self.HEADS               # score projections per node
    @property
    def PAY(self): return self.F2 + 8                 # l2 gather fp8 cols (64|1|3pad|s2'f32)
    @property
    def TW(self): return self.HH                      # whs cols per tile
    @property
    def NG(self): return self.JT // self.GRP


FULL = Cfg()


# --------------------------------------------------------------------------
# Device program
# --------------------------------------------------------------------------

def build_gat_nc(cfg: Cfg, collective: bool = True, iters: int = 1,
                 loop_iters: int = 0, phases: str = "full", debug: bool = False):
    dt = mybir.dt.bfloat16
    f8 = mybir.dt.float8e4
    f32 = mybir.dt.float32
    mult = mybir.AluOpType.mult
    maxop = mybir.AluOpType.max
    bypass = mybir.AluOpType.bypass
    Exp = mybir.ActivationFunctionType.Exp

    N, C, HEADS, F2, R = cfg.N, cfg.C, cfg.HEADS, cfg.F2, cfg.R
    JT, CT, HH, RT = cfg.JT, cfg.CT, cfg.HH, cfg.RT
    S8, PAY, GRP, NG = cfg.S8, cfg.PAY, cfg.GRP, cfg.NG
    TW = cfg.TW
    F2p = F2 + 2
    BETA = 1.0 - ALPHA         # 0.8
    PIPE = 6
    P1_PRE = 8                 # P1 tiles emitted before the slot loop
    P1_PER_SLOT = 2            # P1 tiles interleaved per attention slot

    nc = bacc.Bacc(
        "TRN2", target_bir_lowering=False, debug=False, num_devices=cfg.CORES
    )

    # ---- DRAM I/O -------------------------------------------------------
    # xt: full X^T, global tile order: [:, t*C + ct*128 + jlo]
    xt_d = nc.dram_tensor("xt", [128, JT * C], dt, kind="ExternalInput").ap()
    # xtl: own-shard X^T (for the dedicated G projection): [:, rt*C + ct*128]
    xtl_d = nc.dram_tensor("xtl", [128, RT * C], dt, kind="ExternalInput").ap()
    # mb: mask [j, local i], global tile order: [:, t*R + i]
    mb_d = nc.dram_tensor("mb", [128, JT * R], dt, kind="ExternalInput").ap()
    w1c_d = nc.dram_tensor("w1c", [128, CT * HH], dt,
                           kind="ExternalInput").ap()
    ws1_d = nc.dram_tensor("ws1", [128, CT * S8], dt, kind="ExternalInput").ap()
    w2a_d = nc.dram_tensor("w2a", [128, cfg.CT2 * F2p], dt,
                           kind="ExternalInput").ap()
    id_d = nc.dram_tensor("ident", [128, 128], dt, kind="ExternalInput").ap()
    idf_d = nc.dram_tensor("identf", [128, 128], f32, kind="ExternalInput").ap()
    out_d = nc.dram_tensor("out", [R, F2], f32, kind="ExternalOutput").ap()
    if debug:
        whdbg_d = nc.dram_tensor("whdbg", [128, JT * cfg.TW], dt,
                                 kind="ExternalOutput").ap()
        rqrdbg_d = nc.dram_tensor("rqrdbg", [128, JT * S8], f32,
                                  kind="ExternalOutput").ap()
        rqqdbg_d = nc.dram_tensor("rqqdbg", [128, JT * S8], f32,
                                  kind="ExternalOutput").ap()
        gbdbg_d = nc.dram_tensor("gbdbg", [128, HEADS * R], dt,
                                 kind="ExternalOutput").ap()
        hdbg_d = nc.dram_tensor("hdbg", [128, HEADS * R], dt,
                                kind="ExternalOutput").ap()
        gfdbg_d = nc.dram_tensor("gfdbg", [128, JT * PAY], dt,
                                 kind="ExternalOutput").ap()

    with tile.TileContext(nc) as tc, ExitStack() as ctx:
        const = ctx.enter_context(tc.tile_pool(name="const", bufs=1))
        work = ctx.enter_context(tc.tile_pool(name="work", bufs=4))
        wz = ctx.enter_context(tc.tile_pool(name="wz", bufs=10))
        ps = ctx.enter_context(tc.tile_pool(name="ps", bufs=1, space="PSUM"))
        dram = ctx.enter_context(tc.tile_pool(name="dram", bufs=1, space="DRAM"))

        gsend = dram.tile([128, RT * PAY], f8, name="gsend")
        if cfg.CORES > 4:
            gfull = nc.dram_tensor("gfull_sh", [cfg.CORES * 128, RT * PAY], f8,
                                   addr_space="Shared").ap()
        else:
            gfull = dram.tile([cfg.CORES * 128, RT * PAY], f8, name="gfull")

        import contextlib
        loop_cm = (tc.For_i(0, loop_iters, 1) if loop_iters
                   else contextlib.nullcontext())
        with loop_cm:
          for _it in range(iters):
            # ---- SBUF allocations ---------------------------------------
            xt_sb = const.tile([128, JT * C], dt)
            xtl_sb = const.tile([128, RT * C], dt)
            mb_sb = const.tile([128, JT * R], dt)
            w1c_sb = const.tile([128, CT * HH], dt)
            ws1_sb = const.tile([128, CT * S8], dt)
            w2a_sb = const.tile([128, cfg.CT2 * F2p], dt)
            ident_sb = const.tile([128, 128], dt)
            identf_sb = const.tile([128, 128], f32)
            whs_sb = const.tile([128, JT * TW], dt)     # [Wh_h | 1] x4 per tile
            rqr_sb = const.tile([128, JT * S8], f32)
            rqq_sb = const.tile([128, JT * S8], f32)
            hloc_sb = const.tile([128, HEADS * R], dt)  # h^T (features major)
            gs_sb = const.tile([128, RT * PAY], f8)
            gf_sb = const.tile([128, JT * PAY], f8)
            gfb_sb = const.tile([128, JT * (F2 + 1)], dt)
            rq2r_sb = const.tile([128, JT], f32)
            rq2q_sb = const.tile([128, JT], f32)
            g2row_sb = const.tile([1, R], dt)
            ostage = const.tile([128, RT * F2], f32)

            # ---- input DMAs ---------------------------------------------
            # need-ordered on the sync queue (serial issue cost ~1.6-3us
            # each); the two small G-path weights ride the scalar queue.
            # w1c is deferred: the first aggregation needs it only ~15us in.
            nc.scalar.dma_start(out=ws1_sb, in_=ws1_d)
            nc.scalar.dma_start(out=identf_sb, in_=idf_d)
            nc.sync.dma_start(out=xtl_sb, in_=xtl_d)
            xt_cuts = [0, 4, 12, 22, 32]
            mb_cuts = [0, 4, 12, 22, 32]
            nc.sync.dma_start(out=xt_sb[:, 0: 4 * C], in_=xt_d[:, 0: 4 * C])
            nc.sync.dma_start(out=mb_sb[:, 0: 4 * R], in_=mb_d[:, 0: 4 * R])
            nc.sync.dma_start(out=w1c_sb, in_=w1c_d)
            for q in range(1, 4):
                a, b = xt_cuts[q] * C, xt_cuts[q + 1] * C
                nc.sync.dma_start(out=xt_sb[:, a:b], in_=xt_d[:, a:b])
                a, b = mb_cuts[q] * R, mb_cuts[q + 1] * R
                nc.sync.dma_start(out=mb_sb[:, a:b], in_=mb_d[:, a:b])
            nc.sync.dma_start(out=ident_sb, in_=id_d)
            nc.sync.dma_start(out=w2a_sb, in_=w2a_d)

            ones_col = const.tile([128, 1], dt)
            nc.vector.memset(ones_col, 1.0)

            # ---- G from own shard (dedicated small projection) -----------
            g1row = [const.tile([1, R], dt, name=f"g1row{h}")
                     for h in range(HEADS)]
            pPos = work.tile([128, RT * S8], f32, tag="pPo", name="pPos")
            pPo4 = ps.tile([128, RT * S8], f32, tag="ppj", bufs=1,
                           name="pPo4")
            for rt in range(RT):
                for ct in range(CT):
                    xsl = xtl_sb[:, rt * C + ct * 128: rt * C + (ct + 1) * 128]
                    nc.tensor.matmul(
                        out=pPo4[:, rt * S8: (rt + 1) * S8], lhsT=xsl,
                        rhs=ws1_sb[:, ct * S8: (ct + 1) * S8],
                        start=(rt == 0 and ct == 0), stop=(ct == CT - 1),
                        skip_group_check=True,
                    )
            nc.vector.tensor_copy(out=pPos, in_=pPo4)
            gb = [const.tile([128, R], dt, name=f"g_t{h}")
                  for h in range(HEADS)]

            def emit_g_head(h):
                pth = ps.tile([1, R], f32, tag="sm", bufs=1, name=f"pth{h}")
                for rt in range(RT):
                    nc.tensor.transpose(
                        out=pth[0:1, rt * 128:(rt + 1) * 128],
                        in_=pPos[:, rt * S8 + 2 * h: rt * S8 + 2 * h + 1],
                        identity=identf_sb)
                nc.scalar.activation(out=g1row[h], in_=pth, func=Exp,
                                     scale=BETA)
                nc.gpsimd.partition_broadcast(
                    out_ap=gb[h][:, :], in_ap=g1row[h][0:1, :])

            # ---- P1: full Wh + projections, interleaved with attention ---
            # tiles 0-3 share one proj psum so their r/Q exps are two
            # batched Act ops (Act's serial exp chain gates the first score)
            ppj4 = [None]

            def emit_p1_tile(t):
                pwh = ps.tile([128, HH], f32, tag="pwh", bufs=2,
                              name=f"pwh{t}")
                if t == 0:
                    ppj4[0] = ps.tile([128, 4 * S8], f32, tag="ppj", bufs=1,
                                      name="ppj4")
                if t < 4:
                    ppj = ppj4[0][:, t * S8: (t + 1) * S8]
                    pstart = (t == 0)
                else:
                    ppj = ps.tile([128, S8], f32, tag="ppj", bufs=1,
                                  name=f"ppj{t}")
                    pstart = True
                # proj before Wh: ppj gates scores now; pwh only gates the
                # PIPE-delayed aggregation (and w1c lands later than ws1)
                for ct in range(CT):
                    xsl = xt_sb[:, t * C + ct * 128: t * C + (ct + 1) * 128]
                    nc.tensor.matmul(
                        out=ppj, lhsT=xsl,
                        rhs=ws1_sb[:, ct * S8: (ct + 1) * S8],
                        start=(pstart and ct == 0), stop=(ct == CT - 1),
                        skip_group_check=True,
                    )
                for ct in range(CT):
                    xsl = xt_sb[:, t * C + ct * 128: t * C + (ct + 1) * 128]
                    nc.tensor.matmul(
                        out=pwh, lhsT=xsl,
                        rhs=w1c_sb[:, ct * HH: (ct + 1) * HH],
                        start=(ct == 0), stop=(ct == CT - 1),
                    )
                # r/Q exps first on Act: they gate score production, while
                # the wh copy only gates the (PIPE-delayed) aggregation
                if t == 3:
                    nc.scalar.activation(out=rqr_sb[:, 0: 4 * S8],
                                         in_=ppj4[0], func=Exp, scale=-BETA)
                    nc.scalar.activation(out=rqq_sb[:, 0: 4 * S8],
                                         in_=ppj4[0], func=Exp, scale=1.0)
                elif t >= 4:
                    nc.scalar.activation(out=rqr_sb[:, t * S8:(t + 1) * S8],
                                         in_=ppj, func=Exp, scale=-BETA)
                    nc.scalar.activation(out=rqq_sb[:, t * S8:(t + 1) * S8],
                                         in_=ppj, func=Exp, scale=1.0)
                # GPSIMD cannot read PSUM on real HW: copies go to Act/DVE
                wdst = whs_sb[:, t * TW: (t + 1) * TW]
                if t % 4 == 3:
                    nc.vector.tensor_copy(out=wdst, in_=pwh)
                else:
                    nc.scalar.copy(out=wdst, in_=pwh)

            # ---- attention machinery -------------------------------------
            store = {}
            tt_ctr = [0]

            def issue_scores(key, use_pool, g, rq_r, rq_q, g_in):
                tmp4 = wz.tile([128, GRP * R], dt, tag="tmp", bufs=3,
                               name=f"tmp{key}")
                yg = wz.tile([128, GRP * R], dt, tag="yg", bufs=9,
                             name=f"yg{key}")
                for k in range(GRP):
                    t = GRP * g + k
                    nc.vector.tensor_scalar(
                        out=tmp4[:, k * R: (k + 1) * R], in0=g_in,
                        scalar1=rq_r(t), scalar2=rq_q(t),
                        op0=maxop, op1=mult,
                    )
                eng = nc.gpsimd if use_pool else nc.vector
                eng.tensor_tensor(
                    out=yg, in0=tmp4,
                    in1=mb_sb[:, GRP * g * R: (GRP * g + GRP) * R], op=mult)
                store[key] = yg

            # L1 slot list: head-pair stripes (2 live pagg banks + den bank)
            slots = [(h, g) for hp in range(HEADS // 2) for g in range(NG)
                     for h in (2 * hp, 2 * hp + 1)]
            paggs = {}
            pden = ps.tile([128, HEADS * RT], f32, tag="den", bufs=1,
                           name="pden")

            def issue_matmuls(s):
                h, g = slots[s]
                if g == 0:
                    paggs[h] = ps.tile([128, R], f32, tag="pagg",
                                       bufs=2, name=f"pagg{h}")
                yg = store.pop(("l1", s))
                pA = paggs[h]
                # NOTE: matmul start=True marks the WHOLE 2KB psum bank
                # pending-zero (first write to any pending region overwrites,
                # auto-zeroing it) — so exactly ONE start per bank.
                for k in range(GRP):
                    t = GRP * g + k
                    mv = whs_sb[:, t * TW + h * 128: t * TW + (h + 1) * 128]
                    for c in range(RT):
                        zc = yg[:, k * R + c * 128: k * R + (c + 1) * 128]
                        nc.tensor.matmul(
                            out=pA[:, c * 128: (c + 1) * 128],
                            lhsT=zc, rhs=mv,
                            start=(g == 0 and k == 0 and c == 0),
                            stop=(g == NG - 1 and k == GRP - 1 and c == RT - 1),
                            skip_group_check=True,
                        )
                        nc.tensor.matmul(
                            out=pden[:, h * RT + c: h * RT + c + 1],
                            lhsT=zc, rhs=ones_col,
                            start=(h == 0 and g == 0 and k == 0 and c == 0),
                            stop=(h == HEADS - 1 and g == NG - 1
                                  and k == GRP - 1 and c == RT - 1),
                            skip_group_check=True,
                        )
                return h if g == NG - 1 else None

            pWall = ps.tile([128, RT * F2p], f32, tag="big2", bufs=1,
                            name="pWall")

            def emit_normalize(h):
                pA = paggs.pop(h)
                for c in range(RT):
                    vals = pA[:, c * 128: (c + 1) * 128]
                    den = pden[:, h * RT + c: h * RT + c + 1]
                    rcp = work.tile([128, 1], f32, tag="rcp",
                                    name=f"rcp{h}_{c}")
                    nc.vector.reciprocal(out=rcp, in_=den)
                    hn = work.tile([128, 128], f32, tag="hn",
                                   name=f"hn{h}_{c}")
                    nc.scalar.mul(hn, vals, rcp)
                    eh = work.tile([128, 128], dt, tag="eh",
                                   name=f"eh{h}_{c}")
                    nc.scalar.activation(out=eh, in_=vals, func=Exp,
                                         scale=rcp)
                    htile = work.tile([128, 128], dt, tag="ht",
                                      name=f"ht{h}_{c}")
                    nc.vector._custom_dve(
                        ELU_SEL, out=htile, in0=hn, in1=eh,
                        s0=1.0, s1=0.0, imm2=0.0,
                    )
                    phT = ps.tile([128, 128], dt, tag="sm", bufs=1,
                                  name=f"phT{h}_{c}")
                    nc.tensor.transpose(out=phT, in_=htile,
                                        identity=ident_sb)
                    hdst = hloc_sb[:, h * R + c * 128: h * R + (c + 1) * 128]
                    nc.vector.tensor_copy(out=hdst, in_=phT)
                    nc.tensor.matmul(
                        out=pWall[:, c * F2p: (c + 1) * F2p],
                        lhsT=hdst,
                        rhs=w2a_sb[:, h * F2p: (h + 1) * F2p],
                        start=(h == 0 and c == 0),
                        stop=(h == HEADS - 1 and c == RT - 1),
                        skip_group_check=True,
                    )

            # ---- main L1 loop: scores / P1-interleave / matmuls ----------
            # interleave G-head production with the first P1 tiles so Act's
            # queue alternates g1row exps with early r/Q exps
            p1_next = 0
            emit_g_head(0)
            emit_g_head(1)
            emit_p1_tile(0)
            emit_g_head(2)
            emit_p1_tile(1)
            emit_g_head(3)
            p1_next = 2
            while p1_next < P1_PRE:
                emit_p1_tile(p1_next)
                p1_next += 1
            pending_norm = []
            for s in range(len(slots) + PIPE):
                for _ in range(P1_PER_SLOT):
                    if p1_next < JT:
                        emit_p1_tile(p1_next)
                        p1_next += 1
                if s < len(slots):
                    h, g = slots[s]
                    # ~2/3 of mask-multiplies on GPSIMD (tt there is only
                    # ~1.5x DVE's cost and Pool has queue slack)
                    issue_scores(
                        ("l1", s), (s % 2 == 0), g,
                        lambda t, h=h: rqr_sb[:, t * S8 + 2 * h + 1:
                                              t * S8 + 2 * h + 2],
                        lambda t, h=h: rqq_sb[:, t * S8 + 2 * h + 1:
                                              t * S8 + 2 * h + 2],
                        gb[h])
                if s >= PIPE:
                    done = issue_matmuls(s - PIPE)
                    if done is not None:
                        if done == HEADS - 1:
                            emit_normalize(done)
                        else:
                            pending_norm.append((s + 1, done))
                for due, h in list(pending_norm):
                    if s >= due:
                        emit_normalize(h)
                        pending_norm.remove((due, h))

            if phases == "l1":
                for rt in range(RT):
                    nc.sync.dma_start(out=out_d[rt * 128:(rt + 1) * 128, :],
                                      in_=identf_sb[:, 0:F2])
                continue

            # ---- P5: layer-2 staging + single gather ---------------------
            s1st = const.tile([128, RT], f32, name="s1st")
            for c in range(RT):
                b = c * PAY
                pW = pWall[:, c * F2p: c * F2p + F2]
                nc.scalar.copy(out=gs_sb[:, b: b + F2], in_=pW)
                nc.vector.memset(gs_sb[:, b + F2: b + F2 + 4], 1.0)
                gsf = gs_sb[:, b + F2 + 4: b + PAY].bitcast(f32)
                nc.vector.tensor_copy(
                    out=gsf, in_=pWall[:, c * F2p + F2 + 1: (c + 1) * F2p])
                nc.vector.tensor_copy(
                    out=s1st[:, c: c + 1],
                    in_=pWall[:, c * F2p + F2: c * F2p + F2 + 1])
                pt2 = ps.tile([1, 128], f32, tag="sm", bufs=1,
                              name=f"pt2_{c}")
                nc.tensor.transpose(
                    out=pt2, in_=s1st[:, c: c + 1], identity=identf_sb)
                nc.scalar.activation(
                    out=g2row_sb[0:1, c * 128: (c + 1) * 128], in_=pt2,
                    func=Exp, scale=BETA)
            g2b = const.tile([128, R], dt)
            nc.gpsimd.partition_broadcast(
                out_ap=g2b[:, :], in_ap=g2row_sb[0:1, :])
            nc.sync.dma_start(out=gsend[:, :], in_=gs_sb)
            if collective:
                nc.gpsimd.collective_compute(
                    "AllGather", bypass,
                    replica_groups=[list(range(cfg.CORES))],
                    ins=[gsend.opt()], outs=[gfull.opt()],
                )
            else:
                nc.sync.dma_start(
                    out=gfull.rearrange("(c p) q -> c p q", p=128),
                    in_=gsend[:, :].unsqueeze(0).broadcast_to(
                        (cfg.CORES, 128, RT * PAY)))
            # land gathered payload (global tile order) in two halves so the
            # first score groups start while the second half lands
            gfH = cfg.CORES // 2
            gfF = gf_sb[:, :].bitcast(f32)
            gfF3 = gfF.rearrange("p (t w) -> p t w", t=JT)
            gf3 = gf_sb[:, :].rearrange("p (t w) -> p t w", t=JT)
            gfb3 = gfb_sb[:, :].rearrange("p (t w) -> p t w", t=JT)
            s2c = PAY // 4 - 1
            rq2r3 = rq2r_sb[:, :].rearrange("p (t o) -> p t o", o=1)
            rq2q3 = rq2q_sb[:, :].rearrange("p (t o) -> p t o", o=1)
            for hh in range(2):
                csl = slice(hh * gfH, (hh + 1) * gfH)
                nc.scalar.dma_start(
                    out=gf_sb[:, :].rearrange(
                        "p (c q) -> p c q", c=cfg.CORES)[:, csl, :],
                    in_=gfull.rearrange(
                        "(c p) q -> p c q", p=128)[:, csl, :])
                tsl = slice(hh * JT // 2, (hh + 1) * JT // 2)
                nc.scalar.activation(out=rq2r3[:, tsl, :],
                                     in_=gfF3[:, tsl, s2c: s2c + 1],
                                     func=Exp, scale=-BETA)
                nc.scalar.activation(out=rq2q3[:, tsl, :],
                                     in_=gfF3[:, tsl, s2c: s2c + 1],
                                     func=Exp, scale=1.0)
                # fp8 -> bf16 moving operand for the aggregation matmuls
                nc.scalar.copy(out=gfb3[:, tsl, :],
                               in_=gf3[:, tsl, 0: F2 + 1])

            # ---- P8: layer-2 attention -----------------------------------
            pL2 = ps.tile([128, RT * (F2 + 1)], f32, tag="big2", bufs=1,
                          name="pL2")

            def issue_matmuls2(s):
                g = s
                yg = store.pop(("l2", s))
                for k in range(GRP):
                    t = GRP * g + k
                    mv = gfb_sb[:, t * (F2 + 1): (t + 1) * (F2 + 1)]
                    for c in range(RT):
                        zc = yg[:, k * R + c * 128: k * R + (c + 1) * 128]
                        nc.tensor.matmul(
                            out=pL2[:, c * (F2 + 1): (c + 1) * (F2 + 1)],
                            lhsT=zc, rhs=mv,
                            start=(g == 0 and k == 0 and c == 0),
                            stop=(g == NG - 1 and k == GRP - 1 and c == RT - 1),
                            skip_group_check=True,
                        )

            PIPE2 = 2
            for s in range(NG + PIPE2):
                if s < NG:
                    issue_scores(
                        ("l2", s), (s in (1, 3, 5)), s,
                        lambda t: rq2r_sb[:, t: t + 1],
                        lambda t: rq2q_sb[:, t: t + 1],
                        g2b)
                if s >= PIPE2:
                    issue_matmuls2(s - PIPE2)

            if debug:
                nc.sync.dma_start(out=whdbg_d, in_=whs_sb)
                nc.sync.dma_start(out=rqrdbg_d, in_=rqr_sb)
                nc.sync.dma_start(out=rqqdbg_d, in_=rqq_sb)
                for h in range(HEADS):
                    nc.sync.dma_start(out=gbdbg_d[:, h * R:(h + 1) * R],
                                      in_=gb[h])
                nc.sync.dma_start(out=hdbg_d, in_=hloc_sb)
                nc.sync.dma_start(out=gfdbg_d, in_=gf_sb)

            # ---- P9: finalize (node-major, no transposes) ----------------
            for c in range(RT):
                vals = pL2[:, c * (F2 + 1): c * (F2 + 1) + F2]
                den = pL2[:, c * (F2 + 1) + F2: (c + 1) * (F2 + 1)]
                rc = work.tile([128, 1], f32, tag="rc", name=f"rc{c}")
                nc.vector.reciprocal(out=rc, in_=den)
                nc.scalar.mul(ostage[:, c * F2: (c + 1) * F2], vals, rc)
            nc.sync.dma_start(
                out=out_d.rearrange("(r p) f -> p r f", p=128),
                in_=ostage[:, :].rearrange("p (r f) -> p r f", r=RT))

    nc.compile()
    return nc


# --------------------------------------------------------------------------
# Host-side prep / sharding
# --------------------------------------------------------------------------

def host_prep(cfg: Cfg, g, inputs, W1, a1, W2, a2):
    N, C, H, HEADS, F2, R = cfg.N, cfg.C, cfg.H, cfg.HEADS, cfg.F2, cfg.R
    RT, CT, JT = cfg.RT, cfg.CT, cfg.JT
    X = np.asarray(inputs, np.float32)
    W1 = np.asarray(W1, np.float32)
    a1 = np.asarray(a1, np.float32)
    W2 = np.asarray(W2, np.float32)
    a2 = np.asarray(a2, np.float32)

    def tile128(A):
        # [k*128, cols] row-major -> partition-major [128, k*cols]
        k = A.shape[0] // 128
        return np.ascontiguousarray(
            A.reshape(k, 128, A.shape[1]).transpose(1, 0, 2).reshape(128, -1)
        )

    XT = np.ascontiguousarray(X.T).astype(BF16)                       # [C, N]
    # xt: [128, t*C + ct*128 + jlo]
    xt = np.ascontiguousarray(
        XT.reshape(CT, 128, JT, 128).transpose(1, 2, 0, 3).reshape(128, -1))

    w1c = tile128(np.ascontiguousarray(
        W1.transpose(1, 0, 2).reshape(C, HEADS * H)).astype(BF16))

    # fused score projections: [C, 8] interleaved (s1_h, s2_h)
    ws1_full = np.empty((C, 2 * HEADS), np.float32)
    for h in range(HEADS):
        ws1_full[:, 2 * h] = W1[h] @ a1[h, :H, 0]
        ws1_full[:, 2 * h + 1] = W1[h] @ a1[h, H:, 0]
    ws1 = tile128(ws1_full.astype(BF16))
    w2_full = np.concatenate(
        [W2, W2 @ a2[:F2], W2 @ a2[F2:]], axis=1)                     # [HH, 66]
    w2a = tile128(w2_full.astype(BF16))
    ident = np.eye(128, dtype=BF16)
    identf = np.eye(128, dtype=np.float32)

    adj = np.asarray(g) > 0
    in_maps = []
    for c in range(cfg.CORES):
        rows = slice(c * R, (c + 1) * R)
        mbc = adj[rows].T.astype(BF16)                                # [N, R]
        mbt = tile128(np.ascontiguousarray(mbc))                      # [128, JT*R]
        xl = np.asarray(XT[:, rows])                                  # [C, R]
        xl = xl.reshape(CT, 128, RT, 128).transpose(1, 2, 0, 3).reshape(128, -1)
        in_maps.append({
            "xt": xt,
            "xtl": np.ascontiguousarray(xl),
            "mb": mbt,
            "w1c": w1c, "ws1": ws1, "w2a": w2a,
            "ident": ident, "identf": identf,
        })
    return in_maps


_NC_CACHE = {}


def get_compiled(cfg: Cfg):
    nc = _NC_CACHE.get(cfg)
    if nc is None:
        nc = build_gat_nc(cfg)
        _NC_CACHE[cfg] = nc
    return nc


def kernel(g, inputs, W1, a1, W2, a2):
    cfg = FULL
    nc = get_compiled(cfg)
    in_maps = host_prep(cfg, g, inputs, W1, a1, W2, a2)
    res = run_bass_kernel_spmd(nc, in_maps, core_ids=list(range(cfg.CORES)))
    out = np.concatenate(
        [np.asarray(res.results[c]["out"], np.float32) for c in range(cfg.CORES)],
        axis=0,
    )
    return out
